# revision 3
# baseline (speedup 1.0000x reference)
"""Multi-head attention (B=4, S=2048, D=1024, H=16) on 8 Trainium2 NeuronCores.

Sharding: core (b, g) = batch b, head-group g (8 heads / 512 dims per group).
Each core computes q/k/v projections for its group from the full (transposed)
batch inputs, causal attention for its 8 heads, and a partial output
projection through its 512 rows of wo^T. The host sums the two partial
outputs per batch (standard tensor-parallel unshard) and transposes back.

All matmuls run as float32r (fp32 stored, fp22 multiply, fp32 accumulate).
Layout is fully transposed on-device (d-major) so no on-device transposes are
needed: scores are built as s^T[j, i], softmax normalization is deferred
(flash-style: e @ [v | 1] gives unnormalized out + rowsums in one pass).
"""

import os
import numpy as np

B, S, D = 4, 2048, 1024
H, DK = 16, 64
G = 512            # dims per head-group
P = 128
NCORES = 8

_CACHE = {}
LAST_RESULTS = None  # BassKernelResults of the most recent run (for test harness)


def _build(causal):
    import concourse.bass as bass  # noqa: F401
    import concourse.tile as tile
    from concourse import bacc, mybir

    f32 = mybir.dt.float32
    f32r = mybir.dt.float32r
    Exp = mybir.ActivationFunctionType.Exp
    ADD = mybir.AluOpType.add
    MUL = mybir.AluOpType.mult

    nc = bacc.Bacc(trn_type="TRN2", target_bir_lowering=False, debug=False,
                   num_devices=NCORES)

    def din(name, shape, dt=f32r):
        return nc.dram_tensor(name, shape, dt, kind="ExternalInput").ap()

    xqT = din("xqT", [D, S])
    xkT = din("xkT", [D, S])
    xvT = din("xvT", [D, S])
    wqT = din("wqT", [D, G])
    wkT = din("wkT", [D, G])
    wvT = din("wvT", [D, G])
    woT = din("woT", [G, D])
    bq = din("bq", [G, 1], f32)
    bk = din("bk", [G, 1], f32)
    bv = din("bv", [G, 1], f32)
    bo = din("bo", [D, 1], f32)
    if causal:
        tri = din("tri", [P, P], f32)    # 8*mask^T diagonal block
    else:
        maskT = din("maskT", [S, S])     # 8*mask^T, f32r (fed to matmul)
        identity = din("identity", [P, P])
    out = nc.dram_tensor("out", [D, S], f32, kind="ExternalOutput").ap()

    with tile.TileContext(nc) as tc:
        from contextlib import ExitStack
        with ExitStack() as ctx:
            const = ctx.enter_context(tc.tile_pool(name="const", bufs=1))
            xqp = ctx.enter_context(tc.tile_pool(name="xqp", bufs=8))
            xkp = ctx.enter_context(tc.tile_pool(name="xkp", bufs=8))
            xvp = ctx.enter_context(tc.tile_pool(name="xvp", bufs=8))
            qcol = ctx.enter_context(tc.tile_pool(name="qcol", bufs=5))
            epool = ctx.enter_context(tc.tile_pool(name="ep", bufs=2))
            ocol = ctx.enter_context(tc.tile_pool(name="ocol", bufs=5))
            rpool = ctx.enter_context(tc.tile_pool(name="rp", bufs=4))
            bcpool = ctx.enter_context(tc.tile_pool(name="bcp", bufs=4))
            outp = ctx.enter_context(tc.tile_pool(name="outp", bufs=2))
            pp = ctx.enter_context(tc.tile_pool(name="pp", bufs=2, space="PSUM"))
            pss = ctx.enter_context(tc.tile_pool(name="pss", bufs=2, space="PSUM"))
            pso = ctx.enter_context(tc.tile_pool(name="pso", bufs=2, space="PSUM"))
            if not causal:
                mpool = ctx.enter_context(tc.tile_pool(name="mp", bufs=2))

            # ---- resident constants ----
            wq_sb = [const.tile([P, G], f32r, tag=f"wq{k}", name=f"wq{k}") for k in range(8)]
            wk_sb = [const.tile([P, G], f32r, tag=f"wk{k}", name=f"wk{k}") for k in range(8)]
            wv_sb = [const.tile([P, G], f32r, tag=f"wv{k}", name=f"wv{k}") for k in range(8)]
            wo_sb = [const.tile([P, D], f32r, tag=f"wo{t}", name=f"wo{t}") for t in range(4)]
            for k in range(8):
                nc.sync.dma_start(wq_sb[k][:, :], wqT[k * P:(k + 1) * P, :])
                nc.sync.dma_start(wk_sb[k][:, :], wkT[k * P:(k + 1) * P, :])
                nc.sync.dma_start(wv_sb[k][:, :], wvT[k * P:(k + 1) * P, :])
            for t in range(4):
                nc.sync.dma_start(wo_sb[t][:, :], woT[t * P:(t + 1) * P, :])

            bq_sb = const.tile([P, 4], f32, tag="bq")
            bk_sb = const.tile([P, 4], f32, tag="bk")
            bv_sb = const.tile([P, 4], f32, tag="bv")
            bo_sb = const.tile([P, 8], f32, tag="bo")
            for m in range(4):
                nc.sync.dma_start(bq_sb[:, m:m + 1], bq[m * P:(m + 1) * P, :])
                nc.sync.dma_start(bk_sb[:, m:m + 1], bk[m * P:(m + 1) * P, :])
                nc.sync.dma_start(bv_sb[:, m:m + 1], bv[m * P:(m + 1) * P, :])
            for m in range(8):
                nc.sync.dma_start(bo_sb[:, m:m + 1], bo[m * P:(m + 1) * P, :])

            if causal:
                tri_sb = const.tile([P, P], f32, tag="tri")
                nc.sync.dma_start(tri_sb[:, :], tri[:, :])
            else:
                ident = const.tile([P, P], f32r, tag="ident")
                nc.sync.dma_start(ident[:, :], identity[:, :])

            # k^T resident [512, 2048] as 4 tiles; v resident [2048, 520]
            # as 16 tiles with a ones-column appended per head (for rowsums).
            kt_sb = [const.tile([P, S], f32r, tag=f"kt{m}", name=f"kt{m}") for m in range(4)]
            v_sb = [const.tile([P, 8 * 65], f32r, tag=f"v{j}", name=f"v{j}") for j in range(16)]
            ones_sb = const.tile([P, 8], f32, tag="ones")
            nc.vector.memset(ones_sb[:, :], 1.0)
            ones3 = ones_sb[:, :].rearrange("p (a b) -> p a b", b=1)
            for j in range(16):
                v3 = v_sb[j][:, :].rearrange("p (h x) -> p h x", h=8)
                nc.vector.tensor_copy(v3[:, :, 64:65], ones3)

            for c in range(4):
                # ======== projections for seq chunk c ========
                xq_t = [[xqp.tile([P, 256], f32r, tag="xq", name="xq") for k in range(8)]
                        for h in range(2)]
                xk_t = [[xkp.tile([P, 256], f32r, tag="xk", name="xk") for k in range(8)]
                        for h in range(2)]
                xv_t = [[xvp.tile([P, 256], f32r, tag="xv", name="xv") for k in range(8)]
                        for h in range(2)]
                for h in range(2):
                    c0 = c * 512 + h * 256
                    for k in range(8):
                        nc.sync.dma_start(xq_t[h][k][:, :], xqT[k * P:(k + 1) * P, c0:c0 + 256])
                        nc.sync.dma_start(xk_t[h][k][:, :], xkT[k * P:(k + 1) * P, c0:c0 + 256])
                        nc.sync.dma_start(xv_t[h][k][:, :], xvT[k * P:(k + 1) * P, c0:c0 + 256])

                # q^T column for this chunk: 4 tiles [128, 512]
                qc_tiles = []
                for m in range(4):
                    qc_tiles.append(qcol.tile([P, 512], f32r, tag="qc", name="qc"))
                for h in range(2):
                    for m in range(4):
                        ps = pp.tile([P, 256], f32, tag="pp")
                        for k in range(8):
                            nc.tensor.matmul(ps[:, :], wq_sb[k][:, m * P:(m + 1) * P],
                                             xq_t[h][k][:, :], start=(k == 0), stop=(k == 7))
                        nc.vector.tensor_scalar_add(
                            qc_tiles[m][:, h * 256:(h + 1) * 256], ps[:, :],
                            bq_sb[:, m:m + 1])
                # k^T into resident tiles
                for h in range(2):
                    for m in range(4):
                        ps = pp.tile([P, 256], f32, tag="pp")
                        for k in range(8):
                            nc.tensor.matmul(ps[:, :], wk_sb[k][:, m * P:(m + 1) * P],
                                             xk_t[h][k][:, :], start=(k == 0), stop=(k == 7))
                        c0 = c * 512 + h * 256
                        nc.vector.tensor_scalar_add(
                            kt_sb[m][:, c0:c0 + 256], ps[:, :], bk_sb[:, m:m + 1])
                # v natural layout into interleaved [.|ones] resident tiles
                for q4 in range(4):
                    h, qq = divmod(q4, 2)
                    ps = pp.tile([P, 512], f32, tag="pp")
                    for k in range(8):
                        nc.tensor.matmul(ps[:, :], xv_t[h][k][:, qq * P:(qq + 1) * P],
                                         wv_sb[k][:, :], start=(k == 0), stop=(k == 7))
                    jj = 4 * c + q4
                    v3 = v_sb[jj][:, :].rearrange("p (h x) -> p h x", h=8)
                    p3 = ps[:, :].rearrange("p (h x) -> p h x", h=8)
                    nc.vector.tensor_copy(v3[:, :, 0:64], p3[:, :, :])

                # ======== attention for i-chunk c ========
                jmax = 4 * (c + 1) if causal else 16
                oc_tiles = []
                for p in range(4):  # head pair p: heads 2p (A), 2p+1 (B)
                    oA = pso.tile([65, 512], f32, tag="pso")
                    oB = pso.tile([65, 512], f32, tag="pso")
                    for jj in range(jmax):
                        r = jj - 4 * c
                        a = P * r if (causal and r >= 0) else 0
                        sp = pss.tile([P, 1024], f32, tag="pss")
                        nc.tensor.matmul(sp[:, a:512],
                                         kt_sb[p][0:64, jj * P:(jj + 1) * P],
                                         qc_tiles[p][0:64, a:512],
                                         start=True, stop=causal,
                                         tile_position=(0, 0))
                        nc.tensor.matmul(sp[:, 512 + a:1024],
                                         kt_sb[p][64:128, jj * P:(jj + 1) * P],
                                         qc_tiles[p][64:128, a:512],
                                         start=True, stop=causal,
                                         tile_position=(64, 0))
                        if causal and r >= 0:
                            nc.vector.tensor_tensor(sp[:, a:a + P], sp[:, a:a + P],
                                                    tri_sb[:, :], ADD)
                            nc.vector.tensor_tensor(sp[:, 512 + a:512 + a + P],
                                                    sp[:, 512 + a:512 + a + P],
                                                    tri_sb[:, :], ADD)
                        if not causal:
                            mt = mpool.tile([P, 512], f32r, tag="mt")
                            nc.sync.dma_start(
                                mt[:, :], maskT[jj * P:(jj + 1) * P, c * 512:(c + 1) * 512])
                            nc.tensor.matmul(sp[:, 0:512], ident[:, :], mt[:, :],
                                             start=False, stop=True, skip_group_check=True)
                            nc.tensor.matmul(sp[:, 512:1024], ident[:, :], mt[:, :],
                                             start=False, stop=True, skip_group_check=True)
                        et = epool.tile([P, 1024], f32r, tag="et")
                        sp3 = sp[:, :].rearrange("p (x y) -> p x y", x=2)[:, :, a:512]
                        et3 = et[:, :].rearrange("p (x y) -> p x y", x=2)[:, :, a:512]
                        nc.scalar.activation(et3, sp3, Exp, scale=0.125)
                        nc.tensor.matmul(oA[:, a:512],
                                         v_sb[jj][:, 65 * (2 * p):65 * (2 * p) + 65],
                                         et[:, a:512],
                                         start=(jj == 0), stop=(jj == jmax - 1))
                        nc.tensor.matmul(oB[:, a:512],
                                         v_sb[jj][:, 65 * (2 * p + 1):65 * (2 * p + 1) + 65],
                                         et[:, 512 + a:1024],
                                         start=(jj == 0), stop=(jj == jmax - 1))
                    # normalize: o / rowsum, + bv
                    rA = rpool.tile([1, 512], f32, tag="r")
                    rB = rpool.tile([1, 512], f32, tag="r")
                    nc.vector.reciprocal(rA[:, :], oA[64:65, :])
                    nc.vector.reciprocal(rB[:, :], oB[64:65, :])
                    bA = bcpool.tile([64, 512], f32, tag="bc")
                    bB = bcpool.tile([64, 512], f32, tag="bc")
                    nc.gpsimd.partition_broadcast(bA[:, :], rA[:, :], channels=64)
                    nc.gpsimd.partition_broadcast(bB[:, :], rB[:, :], channels=64)
                    oc = ocol.tile([P, 512], f32r, tag="oc")
                    nc.vector.tensor_tensor(oc[0:64, :], oA[0:64, :], bA[:, :], MUL)
                    nc.vector.tensor_tensor(oc[64:128, :], oB[0:64, :], bB[:, :], MUL)
                    nc.vector.tensor_scalar_add(oc[:, :], oc[:, :], bv_sb[:, p:p + 1])
                    oc_tiles.append(oc)

                # ======== output projection for seq chunk c ========
                for m in range(8):
                    pw = pp.tile([P, 512], f32, tag="pp")
                    for t in range(4):
                        nc.tensor.matmul(pw[:, :], wo_sb[t][:, m * P:(m + 1) * P],
                                         oc_tiles[t][:, :], start=(t == 0), stop=(t == 3))
                    ot = outp.tile([P, 512], f32, tag="ot")
                    nc.vector.tensor_scalar_add(ot[:, :], pw[:, :], bo_sb[:, m:m + 1])
                    nc.sync.dma_start(out[m * P:(m + 1) * P, c * 512:(c + 1) * 512], ot[:, :])

    nc.compile()
    return nc


def _get_nc(causal):
    if causal not in _CACHE:
        _CACHE[causal] = _build(causal)
    return _CACHE[causal]


def kernel(Q, K, V, mask, wq_w, wq_b, wk_w, wk_b, wv_w, wv_b, wo_w, wo_b):
    global LAST_RESULTS
    from concourse.bass_utils import run_bass_kernel_spmd

    Q = np.asarray(Q, np.float32)
    K = np.asarray(K, np.float32)
    V = np.asarray(V, np.float32)
    assert Q.shape == (B, S, D), Q.shape
    m = np.asarray(mask, np.float32).reshape(S, S)

    causal = bool(np.all(np.tril(m) == 0.0)
                  and np.all(m[np.triu_indices(S, 1)] <= -1e8))

    nc = _get_nc(causal)

    if causal:
        tri = np.where(np.arange(P)[None, :] >= np.arange(P)[:, None],
                       np.float32(0.0), np.float32(-8e9))
        tri = np.ascontiguousarray(tri, np.float32)
    else:
        maskT = np.ascontiguousarray(8.0 * m.T, np.float32)

    in_maps = []
    for b in range(B):
        xqT = np.ascontiguousarray(Q[b].T)
        xkT = np.ascontiguousarray(K[b].T)
        xvT = np.ascontiguousarray(V[b].T)
        for g in range(2):
            sl = slice(g * G, (g + 1) * G)
            im = {
                "xqT": xqT, "xkT": xkT, "xvT": xvT,
                "wqT": np.ascontiguousarray(wq_w[sl, :].T),
                "wkT": np.ascontiguousarray(wk_w[sl, :].T),
                "wvT": np.ascontiguousarray(wv_w[sl, :].T),
                "woT": np.ascontiguousarray(wo_w[:, sl].T),
                "bq": np.ascontiguousarray(wq_b[sl].reshape(G, 1), np.float32),
                "bk": np.ascontiguousarray(wk_b[sl].reshape(G, 1), np.float32),
                "bv": np.ascontiguousarray(wv_b[sl].reshape(G, 1), np.float32),
                "bo": (np.ascontiguousarray(wo_b.reshape(D, 1), np.float32)
                       if g == 0 else np.zeros((D, 1), np.float32)),
            }
            if causal:
                im["tri"] = tri
            else:
                im["maskT"] = maskT
                im["identity"] = np.eye(P, dtype=np.float32)
            in_maps.append(im)

    trace = os.environ.get("BASS_KERNEL_TRACE") == "1"
    if trace:
        _install_trace_hook()
    res = run_bass_kernel_spmd(nc, in_maps, core_ids=list(range(NCORES)),
                               trace=trace)
    LAST_RESULTS = res

    outf = np.empty((B, S, D), np.float32)
    for b in range(B):
        part = res.results[2 * b]["out"] + res.results[2 * b + 1]["out"]
        outf[b] = part.T
    return outf


def _install_trace_hook():
    """Register the axon NTFF profiling hook (missing antenv.axon_hooks shim)."""
    import sys
    import types
    import antenv
    if "antenv.axon_hooks" not in sys.modules:
        mod = types.ModuleType("antenv.axon_hooks")
        holder = [None]
        mod.set_axon_ntff_profile_hook = lambda h: holder.__setitem__(0, h)
        mod.get_axon_ntff_profile_hook = lambda: holder[0]
        sys.modules["antenv.axon_hooks"] = mod
        antenv.axon_hooks = mod
        from trn_agent_boot.trn_boot import _ntff_profile_via_ctypes
        mod.set_axon_ntff_profile_hook(
            _ntff_profile_via_ctypes("/opt/axon/libaxon_pjrt.so"))
    import concourse.bass_utils as bu
    bu.upload_artifacts = lambda d: d  # no artifact bucket in this container


# revision 9
# speedup vs baseline: 1.3131x; 1.3131x over previous
"""Multi-head attention (B=4, S=2048, D=1024, H=16) on 8 Trainium2 NeuronCores.

Sharding: core (b, g) = batch b, head-group g (8 heads / 512 dims per group).
Each core computes q/k/v projections for its group from the full (transposed)
batch inputs, causal attention for its 8 heads, and a partial output
projection through its 512 rows of wo^T. The host sums the two partial
outputs per batch (standard tensor-parallel unshard) and transposes back.

Matmuls run in bf16 (fp32 accumulate in PSUM); everything else (softmax
normalization, biases, output) stays fp32. Layout is fully transposed
on-device (d-major) so no on-device transposes are needed: scores are built
as s^T[j, i], softmax normalization is deferred (flash-style: e @ [v | 1]
gives unnormalized out + rowsums in one pass).
"""

import os
import numpy as np

B, S, D = 4, 2048, 1024
H, DK = 16, 64
G = 512            # dims per head-group
P = 128
NCORES = 8

_CACHE = {}
LAST_RESULTS = None  # BassKernelResults of the most recent run (for test harness)


def _build(causal, use_f32r):
    import concourse.bass as bass  # noqa: F401
    import concourse.tile as tile
    from concourse import bacc, mybir

    f32 = mybir.dt.float32
    cdt = mybir.dt.float32r if use_f32r else mybir.dt.bfloat16
    Exp = mybir.ActivationFunctionType.Exp
    ADD = mybir.AluOpType.add
    MUL = mybir.AluOpType.mult

    nc = bacc.Bacc(trn_type="TRN2", target_bir_lowering=False, debug=False,
                   num_devices=NCORES)

    def din(name, shape, dt=cdt):
        return nc.dram_tensor(name, shape, dt, kind="ExternalInput").ap()

    xqT = din("xqT", [D, S])
    xkT = din("xkT", [D, S])
    xvT = din("xvT", [D, S])
    wqT = din("wqT", [D, G])
    wkT = din("wkT", [D, G])
    wvT = din("wvT", [D, G])
    woT = din("woT", [G, D])
    bq = din("bq", [G, 1], f32)
    bk = din("bk", [G, 1], f32)
    bv = din("bv", [G, 1], f32)
    bo = din("bo", [D, 1], f32)
    if causal:
        tri = din("tri", [P, P], f32)    # 8*mask^T diagonal block
    else:
        maskT = din("maskT", [S, S])     # 8*mask^T (fed to matmul via identity)
        identity = din("identity", [P, P])
    out = nc.dram_tensor("out", [D, S], f32, kind="ExternalOutput").ap()

    with tile.TileContext(nc) as tc:
        from contextlib import ExitStack
        with ExitStack() as ctx:
            const = ctx.enter_context(tc.tile_pool(name="const", bufs=1))
            xqp = ctx.enter_context(tc.tile_pool(name="xqp", bufs=16))
            xkp = ctx.enter_context(tc.tile_pool(name="xkp", bufs=16))
            xvp = ctx.enter_context(tc.tile_pool(name="xvp", bufs=16))
            qcol = ctx.enter_context(tc.tile_pool(name="qcol", bufs=8))
            epool = ctx.enter_context(tc.tile_pool(name="ep", bufs=4))
            oprep = ctx.enter_context(tc.tile_pool(name="oprep", bufs=5))
            ocol = ctx.enter_context(tc.tile_pool(name="ocol", bufs=8))
            rpool = ctx.enter_context(tc.tile_pool(name="rp", bufs=3))
            bcpool = ctx.enter_context(tc.tile_pool(name="bcp", bufs=4))
            outp = ctx.enter_context(tc.tile_pool(name="outp", bufs=4))
            pp = ctx.enter_context(tc.tile_pool(name="pp", bufs=2, space="PSUM"))
            pss = ctx.enter_context(tc.tile_pool(name="pss", bufs=2, space="PSUM"))
            pso = ctx.enter_context(tc.tile_pool(name="pso", bufs=2, space="PSUM"))
            drp = ctx.enter_context(tc.tile_pool(name="drp", bufs=2, space="DRAM"))
            if not causal:
                mpool = ctx.enter_context(tc.tile_pool(name="mp", bufs=4))

            # ---- resident constants ----
            wq_sb = [const.tile([P, G], cdt, tag=f"wq{k}", name=f"wq{k}") for k in range(8)]
            wk_sb = [const.tile([P, G], cdt, tag=f"wk{k}", name=f"wk{k}") for k in range(8)]
            wv_sb = [const.tile([P, G], cdt, tag=f"wv{k}", name=f"wv{k}") for k in range(8)]
            wo_sb = [const.tile([P, D], cdt, tag=f"wo{t}", name=f"wo{t}") for t in range(4)]
            for k in range(8):
                nc.sync.dma_start(wq_sb[k][:, :], wqT[k * P:(k + 1) * P, :])
                nc.sync.dma_start(wk_sb[k][:, :], wkT[k * P:(k + 1) * P, :])
                nc.sync.dma_start(wv_sb[k][:, :], wvT[k * P:(k + 1) * P, :])
            for t in range(4):
                nc.sync.dma_start(wo_sb[t][:, :], woT[t * P:(t + 1) * P, :])

            bq_sb = const.tile([P, 4], f32, tag="bq")
            bk_sb = const.tile([P, 4], f32, tag="bk")
            bv_sb = const.tile([P, 4], f32, tag="bv")
            bo_sb = const.tile([P, 8], f32, tag="bo")
            for m in range(4):
                nc.sync.dma_start(bq_sb[:, m:m + 1], bq[m * P:(m + 1) * P, :])
                nc.sync.dma_start(bk_sb[:, m:m + 1], bk[m * P:(m + 1) * P, :])
                nc.sync.dma_start(bv_sb[:, m:m + 1], bv[m * P:(m + 1) * P, :])
            for m in range(8):
                nc.sync.dma_start(bo_sb[:, m:m + 1], bo[m * P:(m + 1) * P, :])

            if causal:
                tri_sb = const.tile([P, P], f32, tag="tri")
                nc.sync.dma_start(tri_sb[:, :], tri[:, :])
            else:
                ident = const.tile([P, P], cdt, tag="ident")
                nc.sync.dma_start(ident[:, :], identity[:, :])

            # k^T resident [512, 2048] as 4 tiles; v resident [2048, 520]
            # as 16 tiles with a ones-column appended per head (for rowsums).
            kt_sb = [const.tile([P, S], cdt, tag=f"kt{m}", name=f"kt{m}") for m in range(4)]
            v_sb = [const.tile([P, 8 * 65], cdt, tag=f"v{j}", name=f"v{j}") for j in range(16)]
            ones_sb = const.tile([P, 8], f32, tag="ones")
            nc.vector.memset(ones_sb[:, :], 1.0)
            ones3 = ones_sb[:, :].rearrange("p (a b) -> p a b", b=1)
            for j in range(16):
                v3 = v_sb[j][:, :].rearrange("p (h x) -> p h x", h=8)
                nc.vector.tensor_copy(v3[:, :, 64:65], ones3)

            for c in range(4):
                # ======== projections for seq chunk c ========
                xq_t = [[xqp.tile([P, 256], cdt, tag="xq", name="xq") for k in range(8)]
                        for h in range(2)]
                xk_t = [[xkp.tile([P, 256], cdt, tag="xk", name="xk") for k in range(8)]
                        for h in range(2)]
                xv_t = [[xvp.tile([P, 256], cdt, tag="xv", name="xv") for k in range(8)]
                        for h in range(2)]
                for h in range(2):
                    c0 = c * 512 + h * 256
                    for k in range(8):
                        nc.sync.dma_start(xq_t[h][k][:, :], xqT[k * P:(k + 1) * P, c0:c0 + 256])
                        nc.sync.dma_start(xk_t[h][k][:, :], xkT[k * P:(k + 1) * P, c0:c0 + 256])
                        nc.sync.dma_start(xv_t[h][k][:, :], xvT[k * P:(k + 1) * P, c0:c0 + 256])

                # q^T column for this chunk: 4 tiles [128, 512]
                qc_tiles = []
                for m in range(4):
                    qc_tiles.append(qcol.tile([P, 512], cdt, tag="qc", name="qc"))
                for h in range(2):
                    for m in range(4):
                        ps = pp.tile([P, 256], f32, tag="pp")
                        for k in range(8):
                            nc.tensor.matmul(ps[:, :], wq_sb[k][:, m * P:(m + 1) * P],
                                             xq_t[h][k][:, :], start=(k == 0), stop=(k == 7))
                        nc.vector.tensor_scalar_add(
                            qc_tiles[m][:, h * 256:(h + 1) * 256], ps[:, :],
                            bq_sb[:, m:m + 1])
                # k^T into resident tiles
                for h in range(2):
                    for m in range(4):
                        ps = pp.tile([P, 256], f32, tag="pp")
                        for k in range(8):
                            nc.tensor.matmul(ps[:, :], wk_sb[k][:, m * P:(m + 1) * P],
                                             xk_t[h][k][:, :], start=(k == 0), stop=(k == 7))
                        c0 = c * 512 + h * 256
                        nc.vector.tensor_scalar_add(
                            kt_sb[m][:, c0:c0 + 256], ps[:, :], bk_sb[:, m:m + 1])
                # v natural layout into interleaved [.|ones] resident tiles
                for q4 in range(4):
                    h, qq = divmod(q4, 2)
                    ps = pp.tile([P, 512], f32, tag="pp")
                    for k in range(8):
                        nc.tensor.matmul(ps[:, :], xv_t[h][k][:, qq * P:(qq + 1) * P],
                                         wv_sb[k][:, :], start=(k == 0), stop=(k == 7))
                    jj = 4 * c + q4
                    v3 = v_sb[jj][:, :].rearrange("p (h x) -> p h x", h=8)
                    p3 = ps[:, :].rearrange("p (h x) -> p h x", h=8)
                    nc.vector.tensor_copy(v3[:, :, 0:64], p3[:, :, :])

                # ======== attention for i-chunk c ========
                jmax = 4 * (c + 1) if causal else 16
                rs_sb = rpool.tile([8, 512], f32, tag="rs")
                op_tiles = []
                for p in range(4):  # head pair p: heads 2p (A), 2p+1 (B)
                    oA = pso.tile([65, 512], f32, tag="pso", name="oA")
                    oB = pso.tile([65, 512], f32, tag="pso", name="oB")
                    for jj in range(jmax):
                        r = jj - 4 * c
                        a = P * r if (causal and r >= 0) else 0
                        sp = pss.tile([P, 1024], f32, tag="pss", name="sp")
                        nc.tensor.matmul(sp[:, a:512],
                                         kt_sb[p][0:64, jj * P:(jj + 1) * P],
                                         qc_tiles[p][0:64, a:512],
                                         start=True, stop=causal,
                                         tile_position=(0, 0))
                        nc.tensor.matmul(sp[:, 512 + a:1024],
                                         kt_sb[p][64:128, jj * P:(jj + 1) * P],
                                         qc_tiles[p][64:128, a:512],
                                         start=True, stop=causal,
                                         tile_position=(64, 0))
                        if causal and r >= 0:
                            nc.vector.tensor_tensor(sp[:, a:a + P], sp[:, a:a + P],
                                                    tri_sb[:, :], ADD)
                            nc.vector.tensor_tensor(sp[:, 512 + a:512 + a + P],
                                                    sp[:, 512 + a:512 + a + P],
                                                    tri_sb[:, :], ADD)
                        if not causal:
                            mt = mpool.tile([P, 512], cdt, tag="mt", name="mt")
                            nc.sync.dma_start(
                                mt[:, :], maskT[jj * P:(jj + 1) * P, c * 512:(c + 1) * 512])
                            nc.tensor.matmul(sp[:, 0:512], ident[:, :], mt[:, :],
                                             start=False, stop=True, skip_group_check=True)
                            nc.tensor.matmul(sp[:, 512:1024], ident[:, :], mt[:, :],
                                             start=False, stop=True, skip_group_check=True)
                        et = epool.tile([P, 1024], cdt, tag="et", name="et")
                        sp3 = sp[:, :].rearrange("p (x y) -> p x y", x=2)[:, :, a:512]
                        et3 = et[:, :].rearrange("p (x y) -> p x y", x=2)[:, :, a:512]
                        nc.scalar.activation(et3, sp3, Exp, scale=0.125)
                        nc.tensor.matmul(oA[:, a:512],
                                         v_sb[jj][:, 65 * (2 * p):65 * (2 * p) + 65],
                                         et[:, a:512],
                                         start=(jj == 0), stop=(jj == jmax - 1))
                        nc.tensor.matmul(oB[:, a:512],
                                         v_sb[jj][:, 65 * (2 * p + 1):65 * (2 * p + 1) + 65],
                                         et[:, 512 + a:1024],
                                         start=(jj == 0), stop=(jj == jmax - 1))
                    # move unnormalized out + rowsums to SBUF, free PSUM.
                    # Rowsums need a partition remap (64 -> 2p) - only DMA
                    # can move data across partitions.
                    opreA = oprep.tile([65, 512], f32, tag="opA", name="opA")
                    opreB = oprep.tile([65, 512], f32, tag="opB", name="opB")
                    nc.vector.tensor_copy(opreA[:, :], oA[:, :])
                    nc.vector.tensor_copy(opreB[:, :], oB[:, :])
                    nc.sync.dma_start(rs_sb[2 * p:2 * p + 1, :], opreA[64:65, :])
                    nc.sync.dma_start(rs_sb[2 * p + 1:2 * p + 2, :], opreB[64:65, :])
                    op_tiles.append((opreA, opreB))
                # one batched reciprocal for all 8 heads of this chunk;
                # bounce through DRAM so stride-0 partition-broadcast DMAs work
                rinv = rpool.tile([8, 512], f32, tag="ri")
                nc.vector.reciprocal(rinv[:, :], rs_sb[:, :])
                rdram = drp.tile([8, 512], f32, tag="rd", name="rd")
                nc.sync.dma_start(rdram[:, :], rinv[:, :])
                oc_tiles = []
                for p in range(4):
                    opreA, opreB = op_tiles[p]
                    bA = bcpool.tile([64, 512], f32, tag="bc", name="bA")
                    bB = bcpool.tile([64, 512], f32, tag="bc", name="bB")
                    nc.gpsimd.dma_start(bA[:, :], rdram[2 * p:2 * p + 1, :].to_broadcast([64, 512]))
                    nc.gpsimd.dma_start(bB[:, :], rdram[2 * p + 1:2 * p + 2, :].to_broadcast([64, 512]))
                    oc = ocol.tile([P, 512], cdt, tag="oc", name="oc")
                    nc.vector.tensor_tensor(oc[0:64, :], opreA[0:64, :], bA[:, :], MUL)
                    # B half lands on partitions 64:128 - needs a DMA hop
                    ocBt = bcpool.tile([64, 512], cdt, tag="ocBt", name="ocBt")
                    nc.vector.tensor_tensor(ocBt[:, :], opreB[0:64, :], bB[:, :], MUL)
                    nc.sync.dma_start(oc[64:128, :], ocBt[:, :])
                    nc.vector.tensor_scalar_add(oc[:, :], oc[:, :], bv_sb[:, p:p + 1])
                    oc_tiles.append(oc)

                # ======== output projection for seq chunk c ========
                for m in range(8):
                    pw = pp.tile([P, 512], f32, tag="pp")
                    for t in range(4):
                        nc.tensor.matmul(pw[:, :], wo_sb[t][:, m * P:(m + 1) * P],
                                         oc_tiles[t][:, :], start=(t == 0), stop=(t == 3))
                    ot = outp.tile([P, 512], f32, tag="ot")
                    nc.vector.tensor_scalar_add(ot[:, :], pw[:, :], bo_sb[:, m:m + 1])
                    nc.sync.dma_start(out[m * P:(m + 1) * P, c * 512:(c + 1) * 512], ot[:, :])

    nc.compile()
    return nc


def _get_nc(causal, use_f32r):
    key = (causal, use_f32r)
    if key not in _CACHE:
        _CACHE[key] = _build(causal, use_f32r)
    return _CACHE[key]


def kernel(Q, K, V, mask, wq_w, wq_b, wk_w, wk_b, wv_w, wv_b, wo_w, wo_b):
    global LAST_RESULTS
    from concourse.bass_utils import run_bass_kernel_spmd

    Q = np.asarray(Q, np.float32)
    K = np.asarray(K, np.float32)
    V = np.asarray(V, np.float32)
    assert Q.shape == (B, S, D), Q.shape
    m = np.asarray(mask, np.float32).reshape(S, S)

    causal = bool(np.all(np.tril(m) == 0.0)
                  and np.all(m[np.triu_indices(S, 1)] <= -1e8))
    use_f32r = os.environ.get("BASS_KERNEL_F32R") == "1"

    if use_f32r:
        def cvt(a):
            return np.ascontiguousarray(np.asarray(a, np.float32))
    else:
        import ml_dtypes

        def cvt(a):
            return np.ascontiguousarray(
                np.asarray(a, np.float32).astype(ml_dtypes.bfloat16))

    nc = _get_nc(causal, use_f32r)

    if causal:
        tri = np.where(np.arange(P)[None, :] >= np.arange(P)[:, None],
                       np.float32(0.0), np.float32(-8e9))
        tri = np.ascontiguousarray(tri, np.float32)
    else:
        maskT = cvt(np.clip(8.0 * m.T, -3e38, 3e38))
        ident_np = cvt(np.eye(P, dtype=np.float32))

    in_maps = []
    for b in range(B):
        xqT = cvt(Q[b].T)
        xkT = cvt(K[b].T)
        xvT = cvt(V[b].T)
        for g in range(2):
            sl = slice(g * G, (g + 1) * G)
            im = {
                "xqT": xqT, "xkT": xkT, "xvT": xvT,
                "wqT": cvt(np.asarray(wq_w)[sl, :].T),
                "wkT": cvt(np.asarray(wk_w)[sl, :].T),
                "wvT": cvt(np.asarray(wv_w)[sl, :].T),
                "woT": cvt(np.asarray(wo_w)[:, sl].T),
                "bq": np.ascontiguousarray(np.asarray(wq_b)[sl].reshape(G, 1), np.float32),
                "bk": np.ascontiguousarray(np.asarray(wk_b)[sl].reshape(G, 1), np.float32),
                "bv": np.ascontiguousarray(np.asarray(wv_b)[sl].reshape(G, 1), np.float32),
                "bo": (np.ascontiguousarray(np.asarray(wo_b).reshape(D, 1), np.float32)
                       if g == 0 else np.zeros((D, 1), np.float32)),
            }
            if causal:
                im["tri"] = tri
            else:
                im["maskT"] = maskT
                im["identity"] = ident_np
            in_maps.append(im)

    trace = os.environ.get("BASS_KERNEL_TRACE") == "1"
    if trace:
        _install_trace_hook()
    res = run_bass_kernel_spmd(nc, in_maps, core_ids=list(range(NCORES)),
                               trace=trace)
    LAST_RESULTS = res

    outf = np.empty((B, S, D), np.float32)
    for b in range(B):
        part = res.results[2 * b]["out"] + res.results[2 * b + 1]["out"]
        outf[b] = part.T
    return outf


def _install_trace_hook():
    """Register the axon NTFF profiling hook (missing antenv.axon_hooks shim)."""
    import sys
    import types
    import antenv
    if "antenv.axon_hooks" not in sys.modules:
        mod = types.ModuleType("antenv.axon_hooks")
        holder = [None]
        mod.set_axon_ntff_profile_hook = lambda h: holder.__setitem__(0, h)
        mod.get_axon_ntff_profile_hook = lambda: holder[0]
        sys.modules["antenv.axon_hooks"] = mod
        antenv.axon_hooks = mod
        from trn_agent_boot.trn_boot import _ntff_profile_via_ctypes
        mod.set_axon_ntff_profile_hook(
            _ntff_profile_via_ctypes("/opt/axon/libaxon_pjrt.so"))
    import concourse.bass_utils as bu
    bu.upload_artifacts = lambda d: d  # no artifact bucket in this container


# revision 10
# speedup vs baseline: 1.6352x; 1.2453x over previous
"""Multi-head attention (B=4, S=2048, D=1024, H=16) on 8 Trainium2 NeuronCores.

Sharding: core (b, g) = batch b, head-group g (8 heads / 512 dims per group).
Each core computes q/k/v projections for its group from the full (transposed)
batch inputs, causal attention for its 8 heads, and a partial output
projection through its 512 rows of wo^T. The host sums the two partial
outputs per batch (standard tensor-parallel unshard) and transposes back.

Matmuls run in bf16 (fp32 accumulate in PSUM); everything else (softmax
normalization, biases, output) stays fp32. Layout is fully transposed
on-device (d-major) so no on-device transposes are needed: scores are built
as s^T[j, i], softmax normalization is deferred (flash-style: e @ [v | 1]
gives unnormalized out + rowsums in one pass).
"""

import os
import numpy as np

B, S, D = 4, 2048, 1024
H, DK = 16, 64
G = 512            # dims per head-group
P = 128
NCORES = 8

_CACHE = {}
LAST_RESULTS = None  # BassKernelResults of the most recent run (for test harness)


def _build(causal, use_f32r):
    import concourse.bass as bass  # noqa: F401
    import concourse.tile as tile
    from concourse import bacc, mybir

    f32 = mybir.dt.float32
    cdt = mybir.dt.float32r if use_f32r else mybir.dt.bfloat16
    Exp = mybir.ActivationFunctionType.Exp
    ADD = mybir.AluOpType.add
    MUL = mybir.AluOpType.mult

    nc = bacc.Bacc(trn_type="TRN2", target_bir_lowering=False, debug=False,
                   num_devices=NCORES)

    def din(name, shape, dt=cdt):
        return nc.dram_tensor(name, shape, dt, kind="ExternalInput").ap()

    xqT = din("xqT", [D, S])
    xkT = din("xkT", [D, S])
    xvT = din("xvT", [D, S])
    wqT = din("wqT", [D, G])
    wkT = din("wkT", [D, G])
    wvT = din("wvT", [D, G])
    woT = din("woT", [G, D])
    bq = din("bq", [G, 1], f32)
    bk = din("bk", [G, 1], f32)
    bv = din("bv", [G, 1], f32)
    bo = din("bo", [D, 1], f32)
    if causal:
        tri = din("tri", [P, P], f32)    # 8*mask^T diagonal block
    else:
        maskT = din("maskT", [S, S])     # 8*mask^T (fed to matmul via identity)
        identity = din("identity", [P, P])
    out = nc.dram_tensor("out", [D, S], f32, kind="ExternalOutput").ap()

    with tile.TileContext(nc) as tc:
        from contextlib import ExitStack
        with ExitStack() as ctx:
            const = ctx.enter_context(tc.tile_pool(name="const", bufs=1))
            xqp = ctx.enter_context(tc.tile_pool(name="xqp", bufs=16))
            xkp = ctx.enter_context(tc.tile_pool(name="xkp", bufs=16))
            xvp = ctx.enter_context(tc.tile_pool(name="xvp", bufs=16))
            qcol = ctx.enter_context(tc.tile_pool(name="qcol", bufs=8))
            epool = ctx.enter_context(tc.tile_pool(name="ep", bufs=4))
            oprep = ctx.enter_context(tc.tile_pool(name="oprep", bufs=5))
            ocol = ctx.enter_context(tc.tile_pool(name="ocol", bufs=8))
            rpool = ctx.enter_context(tc.tile_pool(name="rp", bufs=3))
            bcpool = ctx.enter_context(tc.tile_pool(name="bcp", bufs=4))
            outp = ctx.enter_context(tc.tile_pool(name="outp", bufs=4))
            pp = ctx.enter_context(tc.tile_pool(name="pp", bufs=2, space="PSUM"))
            pss = ctx.enter_context(tc.tile_pool(name="pss", bufs=2, space="PSUM"))
            pso = ctx.enter_context(tc.tile_pool(name="pso", bufs=2, space="PSUM"))
            drp = ctx.enter_context(tc.tile_pool(name="drp", bufs=2, space="DRAM"))
            if not causal:
                mpool = ctx.enter_context(tc.tile_pool(name="mp", bufs=4))

            # ---- resident constants ----
            wq_sb = [const.tile([P, G], cdt, tag=f"wq{k}", name=f"wq{k}") for k in range(8)]
            wk_sb = [const.tile([P, G], cdt, tag=f"wk{k}", name=f"wk{k}") for k in range(8)]
            wv_sb = [const.tile([P, G], cdt, tag=f"wv{k}", name=f"wv{k}") for k in range(8)]
            wo_sb = [const.tile([P, D], cdt, tag=f"wo{t}", name=f"wo{t}") for t in range(4)]
            for k in range(8):
                nc.sync.dma_start(wq_sb[k][:, :], wqT[k * P:(k + 1) * P, :])
                nc.sync.dma_start(wk_sb[k][:, :], wkT[k * P:(k + 1) * P, :])
                nc.sync.dma_start(wv_sb[k][:, :], wvT[k * P:(k + 1) * P, :])
            for t in range(4):
                nc.sync.dma_start(wo_sb[t][:, :], woT[t * P:(t + 1) * P, :])

            bq_sb = const.tile([P, 4], f32, tag="bq")
            bk_sb = const.tile([P, 4], f32, tag="bk")
            bv_sb = const.tile([P, 4], f32, tag="bv")
            bo_sb = const.tile([P, 8], f32, tag="bo")
            for m in range(4):
                nc.sync.dma_start(bq_sb[:, m:m + 1], bq[m * P:(m + 1) * P, :])
                nc.sync.dma_start(bk_sb[:, m:m + 1], bk[m * P:(m + 1) * P, :])
                nc.sync.dma_start(bv_sb[:, m:m + 1], bv[m * P:(m + 1) * P, :])
            for m in range(8):
                nc.sync.dma_start(bo_sb[:, m:m + 1], bo[m * P:(m + 1) * P, :])

            if causal:
                tri_sb = const.tile([P, P], f32, tag="tri")
                nc.sync.dma_start(tri_sb[:, :], tri[:, :])
            else:
                ident = const.tile([P, P], cdt, tag="ident")
                nc.sync.dma_start(ident[:, :], identity[:, :])

            # k^T resident [512, 2048] as 4 tiles; v resident [2048, 520]
            # as 16 tiles with a ones-column appended per head (for rowsums).
            kt_sb = [const.tile([P, S], cdt, tag=f"kt{m}", name=f"kt{m}") for m in range(4)]
            v_sb = [const.tile([P, 8 * 65], cdt, tag=f"v{j}", name=f"v{j}") for j in range(16)]
            ones_sb = const.tile([P, 8], f32, tag="ones")
            nc.vector.memset(ones_sb[:, :], 1.0)
            ones3 = ones_sb[:, :].rearrange("p (a b) -> p a b", b=1)
            for j in range(16):
                v3 = v_sb[j][:, :].rearrange("p (h x) -> p h x", h=8)
                nc.vector.tensor_copy(v3[:, :, 64:65], ones3)

            # ---------- emission helpers ----------
            x_tiles = {}

            def dma_x(c):
                xq_t = [[xqp.tile([P, 256], cdt, tag="xq", name="xq") for k in range(8)]
                        for h in range(2)]
                xk_t = [[xkp.tile([P, 256], cdt, tag="xk", name="xk") for k in range(8)]
                        for h in range(2)]
                xv_t = [[xvp.tile([P, 256], cdt, tag="xv", name="xv") for k in range(8)]
                        for h in range(2)]
                for h in range(2):
                    c0 = c * 512 + h * 256
                    for k in range(8):
                        nc.sync.dma_start(xq_t[h][k][:, :], xqT[k * P:(k + 1) * P, c0:c0 + 256])
                        nc.sync.dma_start(xk_t[h][k][:, :], xkT[k * P:(k + 1) * P, c0:c0 + 256])
                        nc.sync.dma_start(xv_t[h][k][:, :], xvT[k * P:(k + 1) * P, c0:c0 + 256])
                x_tiles[c] = (xq_t, xk_t, xv_t)

            qc_of = {}
            oc_of = {}

            def proj_groups(c):
                """Closures, each emitting one PSUM accumulation group (8 MMs)."""
                xq_t, xk_t, xv_t = x_tiles[c]
                qc_tiles = [qcol.tile([P, 512], cdt, tag="qc", name="qc")
                            for _ in range(4)]
                qc_of[c] = qc_tiles
                groups = []

                def qgrp(h, m):
                    def go():
                        ps = pp.tile([P, 256], f32, tag="pp", name="ps")
                        for k in range(8):
                            nc.tensor.matmul(ps[:, :], wq_sb[k][:, m * P:(m + 1) * P],
                                             xq_t[h][k][:, :], start=(k == 0), stop=(k == 7))
                        nc.vector.tensor_scalar_add(
                            qc_tiles[m][:, h * 256:(h + 1) * 256], ps[:, :],
                            bq_sb[:, m:m + 1])
                    return go

                def kgrp(h, m):
                    def go():
                        ps = pp.tile([P, 256], f32, tag="pp", name="ps")
                        for k in range(8):
                            nc.tensor.matmul(ps[:, :], wk_sb[k][:, m * P:(m + 1) * P],
                                             xk_t[h][k][:, :], start=(k == 0), stop=(k == 7))
                        c0 = c * 512 + h * 256
                        nc.vector.tensor_scalar_add(
                            kt_sb[m][:, c0:c0 + 256], ps[:, :], bk_sb[:, m:m + 1])
                    return go

                def vgrp(q4):
                    def go():
                        h, qq = divmod(q4, 2)
                        ps = pp.tile([P, 512], f32, tag="pp", name="ps")
                        for k in range(8):
                            nc.tensor.matmul(ps[:, :], xv_t[h][k][:, qq * P:(qq + 1) * P],
                                             wv_sb[k][:, :], start=(k == 0), stop=(k == 7))
                        jj = 4 * c + q4
                        v3 = v_sb[jj][:, :].rearrange("p (h x) -> p h x", h=8)
                        p3 = ps[:, :].rearrange("p (h x) -> p h x", h=8)
                        nc.vector.tensor_copy(v3[:, :, 0:64], p3[:, :, :])
                    return go

                for h in range(2):
                    for m in range(4):
                        groups.append(qgrp(h, m))
                for h in range(2):
                    for m in range(4):
                        groups.append(kgrp(h, m))
                for q4 in range(4):
                    groups.append(vgrp(q4))
                return groups

            def wo_groups(c):
                oc_tiles = oc_of[c]
                groups = []

                def wgrp(m):
                    def go():
                        pw = pp.tile([P, 512], f32, tag="pp", name="pw")
                        for t in range(4):
                            nc.tensor.matmul(pw[:, :], wo_sb[t][:, m * P:(m + 1) * P],
                                             oc_tiles[t][:, :], start=(t == 0), stop=(t == 3))
                        ot = outp.tile([P, 512], f32, tag="ot", name="ot")
                        nc.vector.tensor_scalar_add(ot[:, :], pw[:, :], bo_sb[:, m:m + 1])
                        nc.sync.dma_start(out[m * P:(m + 1) * P, c * 512:(c + 1) * 512], ot[:, :])
                    return go

                for m in range(8):
                    groups.append(wgrp(m))
                return groups

            # ---------- schedule ----------
            dma_x(0)
            for g in proj_groups(0):
                g()

            for c in range(4):
                jmax = 4 * (c + 1) if causal else 16
                # fillers: PE work interleaved into attention bubbles
                fillers = []
                if c > 0:
                    fillers += wo_groups(c - 1)
                if c < 3:
                    dma_x(c + 1)
                    fillers += proj_groups(c + 1)
                n_iters = 4 * jmax
                fill_i = 0
                fills_done = 0

                qc_tiles = qc_of[c]
                rs_sb = rpool.tile([8, 512], f32, tag="rs", name="rs")
                op_tiles = []
                for p in range(4):  # head pair p: heads 2p (A), 2p+1 (B)
                    oA = pso.tile([65, 512], f32, tag="pso", name="oA")
                    oB = pso.tile([65, 512], f32, tag="pso", name="oB")
                    for jj in range(jmax):
                        r = jj - 4 * c
                        a = P * r if (causal and r >= 0) else 0
                        sp = pss.tile([P, 1024], f32, tag="pss", name="sp")
                        nc.tensor.matmul(sp[:, a:512],
                                         kt_sb[p][0:64, jj * P:(jj + 1) * P],
                                         qc_tiles[p][0:64, a:512],
                                         start=True, stop=causal,
                                         tile_position=(0, 0))
                        nc.tensor.matmul(sp[:, 512 + a:1024],
                                         kt_sb[p][64:128, jj * P:(jj + 1) * P],
                                         qc_tiles[p][64:128, a:512],
                                         start=True, stop=causal,
                                         tile_position=(64, 0))
                        if causal and r >= 0:
                            nc.vector.tensor_tensor(sp[:, a:a + P], sp[:, a:a + P],
                                                    tri_sb[:, :], ADD)
                            nc.vector.tensor_tensor(sp[:, 512 + a:512 + a + P],
                                                    sp[:, 512 + a:512 + a + P],
                                                    tri_sb[:, :], ADD)
                        if not causal:
                            mt = mpool.tile([P, 512], cdt, tag="mt", name="mt")
                            nc.sync.dma_start(
                                mt[:, :], maskT[jj * P:(jj + 1) * P, c * 512:(c + 1) * 512])
                            nc.tensor.matmul(sp[:, 0:512], ident[:, :], mt[:, :],
                                             start=False, stop=True, skip_group_check=True)
                            nc.tensor.matmul(sp[:, 512:1024], ident[:, :], mt[:, :],
                                             start=False, stop=True, skip_group_check=True)
                        et = epool.tile([P, 1024], cdt, tag="et", name="et")
                        sp3 = sp[:, :].rearrange("p (x y) -> p x y", x=2)[:, :, a:512]
                        et3 = et[:, :].rearrange("p (x y) -> p x y", x=2)[:, :, a:512]
                        nc.scalar.activation(et3, sp3, Exp, scale=0.125)
                        nc.tensor.matmul(oA[:, a:512],
                                         v_sb[jj][:, 65 * (2 * p):65 * (2 * p) + 65],
                                         et[:, a:512],
                                         start=(jj == 0), stop=(jj == jmax - 1))
                        nc.tensor.matmul(oB[:, a:512],
                                         v_sb[jj][:, 65 * (2 * p + 1):65 * (2 * p + 1) + 65],
                                         et[:, 512 + a:1024],
                                         start=(jj == 0), stop=(jj == jmax - 1))
                        # interleave filler PE work (next-chunk proj / prev wo)
                        fill_i += 1
                        want = (fill_i * len(fillers)) // n_iters
                        while fills_done < want:
                            fillers[fills_done]()
                            fills_done += 1
                    # move unnormalized out + rowsums to SBUF, free PSUM.
                    opreA = oprep.tile([65, 512], f32, tag="opA", name="opA")
                    opreB = oprep.tile([65, 512], f32, tag="opB", name="opB")
                    nc.vector.tensor_copy(opreA[:, :], oA[:, :])
                    nc.vector.tensor_copy(opreB[:, :], oB[:, :])
                    nc.sync.dma_start(rs_sb[2 * p:2 * p + 1, :], opreA[64:65, :])
                    nc.sync.dma_start(rs_sb[2 * p + 1:2 * p + 2, :], opreB[64:65, :])
                    op_tiles.append((opreA, opreB))
                while fills_done < len(fillers):
                    fillers[fills_done]()
                    fills_done += 1
                # one batched reciprocal for all 8 heads of this chunk;
                # bounce through DRAM so stride-0 partition-broadcast DMAs work
                rinv = rpool.tile([8, 512], f32, tag="ri")
                nc.vector.reciprocal(rinv[:, :], rs_sb[:, :])
                rdram = drp.tile([8, 512], f32, tag="rd", name="rd")
                nc.sync.dma_start(rdram[:, :], rinv[:, :])
                oc_tiles = []
                for p in range(4):
                    opreA, opreB = op_tiles[p]
                    bA = bcpool.tile([64, 512], f32, tag="bc", name="bA")
                    bB = bcpool.tile([64, 512], f32, tag="bc", name="bB")
                    nc.gpsimd.dma_start(bA[:, :], rdram[2 * p:2 * p + 1, :].to_broadcast([64, 512]))
                    nc.gpsimd.dma_start(bB[:, :], rdram[2 * p + 1:2 * p + 2, :].to_broadcast([64, 512]))
                    oc = ocol.tile([P, 512], cdt, tag="oc", name="oc")
                    nc.vector.tensor_tensor(oc[0:64, :], opreA[0:64, :], bA[:, :], MUL)
                    # B half lands on partitions 64:128 - needs a DMA hop
                    ocBt = bcpool.tile([64, 512], cdt, tag="ocBt", name="ocBt")
                    nc.vector.tensor_tensor(ocBt[:, :], opreB[0:64, :], bB[:, :], MUL)
                    nc.sync.dma_start(oc[64:128, :], ocBt[:, :])
                    nc.vector.tensor_scalar_add(oc[:, :], oc[:, :], bv_sb[:, p:p + 1])
                    oc_tiles.append(oc)
                oc_of[c] = oc_tiles

            for g in wo_groups(3):
                g()

    nc.compile()
    return nc


def _get_nc(causal, use_f32r):
    key = (causal, use_f32r)
    if key not in _CACHE:
        _CACHE[key] = _build(causal, use_f32r)
    return _CACHE[key]


def kernel(Q, K, V, mask, wq_w, wq_b, wk_w, wk_b, wv_w, wv_b, wo_w, wo_b):
    global LAST_RESULTS
    from concourse.bass_utils import run_bass_kernel_spmd

    Q = np.asarray(Q, np.float32)
    K = np.asarray(K, np.float32)
    V = np.asarray(V, np.float32)
    assert Q.shape == (B, S, D), Q.shape
    m = np.asarray(mask, np.float32).reshape(S, S)

    causal = bool(np.all(np.tril(m) == 0.0)
                  and np.all(m[np.triu_indices(S, 1)] <= -1e8))
    use_f32r = os.environ.get("BASS_KERNEL_F32R") == "1"

    if use_f32r:
        def cvt(a):
            return np.ascontiguousarray(np.asarray(a, np.float32))
    else:
        import ml_dtypes

        def cvt(a):
            return np.ascontiguousarray(
                np.asarray(a, np.float32).astype(ml_dtypes.bfloat16))

    nc = _get_nc(causal, use_f32r)

    if causal:
        tri = np.where(np.arange(P)[None, :] >= np.arange(P)[:, None],
                       np.float32(0.0), np.float32(-8e9))
        tri = np.ascontiguousarray(tri, np.float32)
    else:
        maskT = cvt(np.clip(8.0 * m.T, -3e38, 3e38))
        ident_np = cvt(np.eye(P, dtype=np.float32))

    in_maps = []
    for b in range(B):
        xqT = cvt(Q[b].T)
        xkT = cvt(K[b].T)
        xvT = cvt(V[b].T)
        for g in range(2):
            sl = slice(g * G, (g + 1) * G)
            im = {
                "xqT": xqT, "xkT": xkT, "xvT": xvT,
                "wqT": cvt(np.asarray(wq_w)[sl, :].T),
                "wkT": cvt(np.asarray(wk_w)[sl, :].T),
                "wvT": cvt(np.asarray(wv_w)[sl, :].T),
                "woT": cvt(np.asarray(wo_w)[:, sl].T),
                "bq": np.ascontiguousarray(np.asarray(wq_b)[sl].reshape(G, 1), np.float32),
                "bk": np.ascontiguousarray(np.asarray(wk_b)[sl].reshape(G, 1), np.float32),
                "bv": np.ascontiguousarray(np.asarray(wv_b)[sl].reshape(G, 1), np.float32),
                "bo": (np.ascontiguousarray(np.asarray(wo_b).reshape(D, 1), np.float32)
                       if g == 0 else np.zeros((D, 1), np.float32)),
            }
            if causal:
                im["tri"] = tri
            else:
                im["maskT"] = maskT
                im["identity"] = ident_np
            in_maps.append(im)

    trace = os.environ.get("BASS_KERNEL_TRACE") == "1"
    if trace:
        _install_trace_hook()
    res = run_bass_kernel_spmd(nc, in_maps, core_ids=list(range(NCORES)),
                               trace=trace)
    LAST_RESULTS = res

    outf = np.empty((B, S, D), np.float32)
    for b in range(B):
        part = res.results[2 * b]["out"] + res.results[2 * b + 1]["out"]
        outf[b] = part.T
    return outf


def _install_trace_hook():
    """Register the axon NTFF profiling hook (missing antenv.axon_hooks shim)."""
    import sys
    import types
    import antenv
    if "antenv.axon_hooks" not in sys.modules:
        mod = types.ModuleType("antenv.axon_hooks")
        holder = [None]
        mod.set_axon_ntff_profile_hook = lambda h: holder.__setitem__(0, h)
        mod.get_axon_ntff_profile_hook = lambda: holder[0]
        sys.modules["antenv.axon_hooks"] = mod
        antenv.axon_hooks = mod
        from trn_agent_boot.trn_boot import _ntff_profile_via_ctypes
        mod.set_axon_ntff_profile_hook(
            _ntff_profile_via_ctypes("/opt/axon/libaxon_pjrt.so"))
    import concourse.bass_utils as bu
    bu.upload_artifacts = lambda d: d  # no artifact bucket in this container


# revision 11
# speedup vs baseline: 1.9253x; 1.1774x over previous
"""Multi-head attention (B=4, S=2048, D=1024, H=16) on 8 Trainium2 NeuronCores.

Sharding: core (b, g) = batch b, head-group g (8 heads / 512 dims per group).
Each core computes q/k/v projections for its group from the full (transposed)
batch inputs, causal attention for its 8 heads, and a partial output
projection through its 512 rows of wo^T. The host sums the two partial
outputs per batch (standard tensor-parallel unshard) and transposes back.

Matmuls run in bf16 (fp32 accumulate in PSUM); everything else (softmax
normalization, biases, output) stays fp32. Layout is fully transposed
on-device (d-major) so no on-device transposes are needed: scores are built
as s^T[j, i], softmax normalization is deferred (flash-style: e @ [v | 1]
gives unnormalized out + rowsums in one pass).
"""

import os
import numpy as np

B, S, D = 4, 2048, 1024
H, DK = 16, 64
G = 512            # dims per head-group
P = 128
NCORES = 8

_CACHE = {}
LAST_RESULTS = None  # BassKernelResults of the most recent run (for test harness)


def _build(causal, use_f32r):
    import concourse.bass as bass  # noqa: F401
    import concourse.tile as tile
    from concourse import bacc, mybir

    f32 = mybir.dt.float32
    cdt = mybir.dt.float32r if use_f32r else mybir.dt.bfloat16
    Exp = mybir.ActivationFunctionType.Exp
    ADD = mybir.AluOpType.add
    MUL = mybir.AluOpType.mult

    nc = bacc.Bacc(trn_type="TRN2", target_bir_lowering=False, debug=False,
                   num_devices=NCORES)

    def din(name, shape, dt=cdt):
        return nc.dram_tensor(name, shape, dt, kind="ExternalInput").ap()

    xqT = din("xqT", [D, S])
    xkT = din("xkT", [D, S])
    xvT = din("xvT", [D, S])
    wqT = din("wqT", [D, G])
    wkT = din("wkT", [D, G])
    wvT = din("wvT", [D, G])
    woT = din("woT", [G, D])
    bq = din("bq", [G, 1], f32)
    bk = din("bk", [G, 1], f32)
    bv = din("bv", [G, 1], f32)
    bo = din("bo", [D, 1], f32)
    identity = din("identity", [P, P])
    if causal:
        tri = din("tri", [P, P])         # 8*mask^T diagonal block
    else:
        maskT = din("maskT", [S, S])     # 8*mask^T (fed to matmul via identity)
    out = nc.dram_tensor("out", [D, S], f32, kind="ExternalOutput").ap()

    with tile.TileContext(nc) as tc:
        from contextlib import ExitStack
        with ExitStack() as ctx:
            const = ctx.enter_context(tc.tile_pool(name="const", bufs=1))
            xqp = ctx.enter_context(tc.tile_pool(name="xqp", bufs=2))
            xkp = ctx.enter_context(tc.tile_pool(name="xkp", bufs=2))
            xvp = ctx.enter_context(tc.tile_pool(name="xvp", bufs=2))
            qcol = ctx.enter_context(tc.tile_pool(name="qcol", bufs=8))
            epool = ctx.enter_context(tc.tile_pool(name="ep", bufs=4))
            oprep = ctx.enter_context(tc.tile_pool(name="oprep", bufs=5))
            ocol = ctx.enter_context(tc.tile_pool(name="ocol", bufs=8))
            rpool = ctx.enter_context(tc.tile_pool(name="rp", bufs=3))
            bcpool = ctx.enter_context(tc.tile_pool(name="bcp", bufs=4))
            outp = ctx.enter_context(tc.tile_pool(name="outp", bufs=4))
            pp = ctx.enter_context(tc.tile_pool(name="pp", bufs=2, space="PSUM"))
            pss = ctx.enter_context(tc.tile_pool(name="pss", bufs=2, space="PSUM"))
            pso = ctx.enter_context(tc.tile_pool(name="pso", bufs=2, space="PSUM"))
            drp = ctx.enter_context(tc.tile_pool(name="drp", bufs=2, space="DRAM"))
            if not causal:
                mpool = ctx.enter_context(tc.tile_pool(name="mp", bufs=4))

            # ---- resident constants (one DMA per tensor via 3D APs) ----
            wq_sb = const.tile([P, 8, G], cdt, tag="wq", name="wq")
            wk_sb = const.tile([P, 8, G], cdt, tag="wk", name="wk")
            wv_sb = const.tile([P, 8, G], cdt, tag="wv", name="wv")
            wo_sb = const.tile([P, 4, D], cdt, tag="wo", name="wo")
            nc.sync.dma_start(wq_sb[:, :, :], wqT.rearrange("(k p) g -> p k g", p=P))
            nc.sync.dma_start(wk_sb[:, :, :], wkT.rearrange("(k p) g -> p k g", p=P))
            nc.sync.dma_start(wv_sb[:, :, :], wvT.rearrange("(k p) g -> p k g", p=P))
            nc.sync.dma_start(wo_sb[:, :, :], woT.rearrange("(t p) d -> p t d", p=P))

            bq_sb = const.tile([P, 4], f32, tag="bq")
            bk_sb = const.tile([P, 4], f32, tag="bk")
            bv_sb = const.tile([P, 4], f32, tag="bv")
            bo_sb = const.tile([P, 8], f32, tag="bo")
            nc.sync.dma_start(bq_sb[:, :], bq.rearrange("(m p) o -> p (m o)", p=P))
            nc.sync.dma_start(bk_sb[:, :], bk.rearrange("(m p) o -> p (m o)", p=P))
            nc.sync.dma_start(bv_sb[:, :], bv.rearrange("(m p) o -> p (m o)", p=P))
            nc.sync.dma_start(bo_sb[:, :], bo.rearrange("(m p) o -> p (m o)", p=P))

            ident = const.tile([P, P], cdt, tag="ident")
            nc.sync.dma_start(ident[:, :], identity[:, :])
            if causal:
                tri_sb = const.tile([P, P], cdt, tag="tri")
                nc.sync.dma_start(tri_sb[:, :], tri[:, :])

            # k^T resident [512, 2048] as 4 tiles; v resident [2048, 520]
            # as 16 tiles with a ones-column appended per head (for rowsums).
            kt_sb = [const.tile([P, S], cdt, tag=f"kt{m}", name=f"kt{m}") for m in range(4)]
            v_sb = [const.tile([P, 8 * 65], cdt, tag=f"v{j}", name=f"v{j}") for j in range(16)]
            ones_sb = const.tile([P, 8], f32, tag="ones")
            nc.vector.memset(ones_sb[:, :], 1.0)
            ones3 = ones_sb[:, :].rearrange("p (a b) -> p a b", b=1)
            for j in range(16):
                v3 = v_sb[j][:, :].rearrange("p (h x) -> p h x", h=8)
                nc.vector.tensor_copy(v3[:, :, 64:65], ones3)

            # ---------- emission helpers ----------
            x_tiles = {}

            xqT3 = xqT.rearrange("(k p) s -> p k s", p=P)
            xkT3 = xkT.rearrange("(k p) s -> p k s", p=P)
            xvT3 = xvT.rearrange("(k p) s -> p k s", p=P)

            def dma_x(c):
                xq_c = xqp.tile([P, 8, 512], cdt, tag="xq", name="xq")
                xk_c = xkp.tile([P, 8, 512], cdt, tag="xk", name="xk")
                xv_c = xvp.tile([P, 8, 512], cdt, tag="xv", name="xv")
                s0 = c * 512
                nc.sync.dma_start(xq_c[:, :, :], xqT3[:, :, s0:s0 + 512])
                nc.sync.dma_start(xk_c[:, :, :], xkT3[:, :, s0:s0 + 512])
                nc.sync.dma_start(xv_c[:, :, :], xvT3[:, :, s0:s0 + 512])
                x_tiles[c] = (xq_c, xk_c, xv_c)

            qc_of = {}
            oc_of = {}

            def proj_groups(c):
                """Closures, each emitting one PSUM accumulation group (8 MMs)."""
                xq_c, xk_c, xv_c = x_tiles[c]
                qc_tiles = [qcol.tile([P, 512], cdt, tag="qc", name="qc")
                            for _ in range(4)]
                qc_of[c] = qc_tiles
                groups = []

                def qgrp(h, m):
                    def go():
                        ps = pp.tile([P, 256], f32, tag="pp", name="ps")
                        for k in range(8):
                            nc.tensor.matmul(ps[:, :], wq_sb[:, k, m * P:(m + 1) * P],
                                             xq_c[:, k, h * 256:(h + 1) * 256],
                                             start=(k == 0), stop=(k == 7))
                        nc.vector.tensor_scalar_add(
                            qc_tiles[m][:, h * 256:(h + 1) * 256], ps[:, :],
                            bq_sb[:, m:m + 1])
                    return go

                def kgrp(h, m):
                    def go():
                        ps = pp.tile([P, 256], f32, tag="pp", name="ps")
                        for k in range(8):
                            nc.tensor.matmul(ps[:, :], wk_sb[:, k, m * P:(m + 1) * P],
                                             xk_c[:, k, h * 256:(h + 1) * 256],
                                             start=(k == 0), stop=(k == 7))
                        c0 = c * 512 + h * 256
                        nc.vector.tensor_scalar_add(
                            kt_sb[m][:, c0:c0 + 256], ps[:, :], bk_sb[:, m:m + 1])
                    return go

                def vgrp(q4):
                    def go():
                        ps = pp.tile([P, 512], f32, tag="pp", name="ps")
                        for k in range(8):
                            nc.tensor.matmul(ps[:, :], xv_c[:, k, q4 * P:(q4 + 1) * P],
                                             wv_sb[:, k, :], start=(k == 0), stop=(k == 7))
                        jj = 4 * c + q4
                        v3 = v_sb[jj][:, :].rearrange("p (h x) -> p h x", h=8)
                        p3 = ps[:, :].rearrange("p (h x) -> p h x", h=8)
                        nc.vector.tensor_copy(v3[:, :, 0:64], p3[:, :, :])
                    return go

                for h in range(2):
                    for m in range(4):
                        groups.append(qgrp(h, m))
                for h in range(2):
                    for m in range(4):
                        groups.append(kgrp(h, m))
                for q4 in range(4):
                    groups.append(vgrp(q4))
                return groups

            def wo_groups(c):
                oc_tiles = oc_of[c]
                groups = []

                def wgrp(m):
                    def go():
                        pw = pp.tile([P, 512], f32, tag="pp", name="pw")
                        for t in range(4):
                            nc.tensor.matmul(pw[:, :], wo_sb[:, t, m * P:(m + 1) * P],
                                             oc_tiles[t][:, :], start=(t == 0), stop=(t == 3))
                        ot = outp.tile([P, 512], f32, tag="ot", name="ot")
                        nc.vector.tensor_scalar_add(ot[:, :], pw[:, :], bo_sb[:, m:m + 1])
                        nc.sync.dma_start(out[m * P:(m + 1) * P, c * 512:(c + 1) * 512], ot[:, :])
                    return go

                for m in range(8):
                    groups.append(wgrp(m))
                return groups

            # ---------- schedule ----------
            dma_x(0)
            for g in proj_groups(0):
                g()

            for c in range(4):
                jmax = 4 * (c + 1) if causal else 16
                # fillers: PE work interleaved into attention bubbles
                fillers = []
                if c > 0:
                    fillers += wo_groups(c - 1)
                if c < 3:
                    dma_x(c + 1)
                    fillers += proj_groups(c + 1)
                n_iters = 4 * jmax
                fill_i = 0
                fills_done = 0

                qc_tiles = qc_of[c]
                rs_sb = rpool.tile([8, 512], f32, tag="rs", name="rs")
                op_tiles = []
                for p in range(4):  # head pair p: heads 2p (A), 2p+1 (B)
                    oA = pso.tile([65, 512], f32, tag="pso", name="oA")
                    oB = pso.tile([65, 512], f32, tag="pso", name="oB")
                    for jj in range(jmax):
                        r = jj - 4 * c
                        a = P * r if (causal and r >= 0) else 0
                        sp = pss.tile([P, 1024], f32, tag="pss", name="sp")
                        nc.tensor.matmul(sp[:, a:512],
                                         kt_sb[p][0:64, jj * P:(jj + 1) * P],
                                         qc_tiles[p][0:64, a:512],
                                         start=True, stop=causal and r < 0,
                                         tile_position=(0, 0))
                        nc.tensor.matmul(sp[:, 512 + a:1024],
                                         kt_sb[p][64:128, jj * P:(jj + 1) * P],
                                         qc_tiles[p][64:128, a:512],
                                         start=True, stop=causal and r < 0,
                                         tile_position=(64, 0))
                        if causal and r >= 0:
                            nc.tensor.matmul(sp[:, a:a + P], ident[:, :], tri_sb[:, :],
                                             start=False, stop=True, skip_group_check=True)
                            nc.tensor.matmul(sp[:, 512 + a:512 + a + P], ident[:, :],
                                             tri_sb[:, :], start=False, stop=True,
                                             skip_group_check=True)
                        if not causal:
                            mt = mpool.tile([P, 512], cdt, tag="mt", name="mt")
                            nc.sync.dma_start(
                                mt[:, :], maskT[jj * P:(jj + 1) * P, c * 512:(c + 1) * 512])
                            nc.tensor.matmul(sp[:, 0:512], ident[:, :], mt[:, :],
                                             start=False, stop=True, skip_group_check=True)
                            nc.tensor.matmul(sp[:, 512:1024], ident[:, :], mt[:, :],
                                             start=False, stop=True, skip_group_check=True)
                        et = epool.tile([P, 1024], cdt, tag="et", name="et")
                        sp3 = sp[:, :].rearrange("p (x y) -> p x y", x=2)[:, :, a:512]
                        et3 = et[:, :].rearrange("p (x y) -> p x y", x=2)[:, :, a:512]
                        nc.scalar.activation(et3, sp3, Exp, scale=0.125)
                        nc.tensor.matmul(oA[:, a:512],
                                         v_sb[jj][:, 65 * (2 * p):65 * (2 * p) + 65],
                                         et[:, a:512],
                                         start=(jj == 0), stop=(jj == jmax - 1))
                        nc.tensor.matmul(oB[:, a:512],
                                         v_sb[jj][:, 65 * (2 * p + 1):65 * (2 * p + 1) + 65],
                                         et[:, 512 + a:1024],
                                         start=(jj == 0), stop=(jj == jmax - 1))
                        # interleave filler PE work (next-chunk proj / prev wo)
                        fill_i += 1
                        want = (fill_i * len(fillers)) // n_iters
                        while fills_done < want:
                            fillers[fills_done]()
                            fills_done += 1
                    # move unnormalized out + rowsums to SBUF, free PSUM.
                    opreA = oprep.tile([65, 512], f32, tag="opA", name="opA")
                    opreB = oprep.tile([65, 512], f32, tag="opB", name="opB")
                    nc.vector.tensor_copy(opreA[:, :], oA[:, :])
                    nc.vector.tensor_copy(opreB[:, :], oB[:, :])
                    nc.gpsimd.dma_start(rs_sb[2 * p:2 * p + 1, :], opreA[64:65, :])
                    nc.gpsimd.dma_start(rs_sb[2 * p + 1:2 * p + 2, :], opreB[64:65, :])
                    op_tiles.append((opreA, opreB))
                while fills_done < len(fillers):
                    fillers[fills_done]()
                    fills_done += 1
                # one batched reciprocal for all 8 heads of this chunk;
                # bounce through DRAM so stride-0 partition-broadcast DMAs work
                rinv = rpool.tile([8, 512], f32, tag="ri")
                nc.vector.reciprocal(rinv[:, :], rs_sb[:, :])
                rdram = drp.tile([8, 512], f32, tag="rd", name="rd")
                nc.gpsimd.dma_start(rdram[:, :], rinv[:, :])
                oc_tiles = []
                for p in range(4):
                    opreA, opreB = op_tiles[p]
                    bA = bcpool.tile([64, 512], f32, tag="bc", name="bA")
                    bB = bcpool.tile([64, 512], f32, tag="bc", name="bB")
                    nc.gpsimd.dma_start(bA[:, :], rdram[2 * p:2 * p + 1, :].to_broadcast([64, 512]))
                    nc.gpsimd.dma_start(bB[:, :], rdram[2 * p + 1:2 * p + 2, :].to_broadcast([64, 512]))
                    oc = ocol.tile([P, 512], cdt, tag="oc", name="oc")
                    nc.vector.tensor_tensor(oc[0:64, :], opreA[0:64, :], bA[:, :], MUL)
                    # B half lands on partitions 64:128 - needs a DMA hop
                    ocBt = bcpool.tile([64, 512], cdt, tag="ocBt", name="ocBt")
                    nc.vector.tensor_tensor(ocBt[:, :], opreB[0:64, :], bB[:, :], MUL)
                    nc.gpsimd.dma_start(oc[64:128, :], ocBt[:, :])
                    nc.vector.tensor_scalar_add(oc[:, :], oc[:, :], bv_sb[:, p:p + 1])
                    oc_tiles.append(oc)
                oc_of[c] = oc_tiles

            for g in wo_groups(3):
                g()

    nc.compile()
    return nc


def _get_nc(causal, use_f32r):
    key = (causal, use_f32r)
    if key not in _CACHE:
        _CACHE[key] = _build(causal, use_f32r)
    return _CACHE[key]


def kernel(Q, K, V, mask, wq_w, wq_b, wk_w, wk_b, wv_w, wv_b, wo_w, wo_b):
    global LAST_RESULTS
    from concourse.bass_utils import run_bass_kernel_spmd

    Q = np.asarray(Q, np.float32)
    K = np.asarray(K, np.float32)
    V = np.asarray(V, np.float32)
    assert Q.shape == (B, S, D), Q.shape
    m = np.asarray(mask, np.float32).reshape(S, S)

    causal = bool(np.all(np.tril(m) == 0.0)
                  and np.all(m[np.triu_indices(S, 1)] <= -1e8))
    use_f32r = os.environ.get("BASS_KERNEL_F32R") == "1"

    if use_f32r:
        def cvt(a):
            return np.ascontiguousarray(np.asarray(a, np.float32))
    else:
        import ml_dtypes

        def cvt(a):
            return np.ascontiguousarray(
                np.asarray(a, np.float32).astype(ml_dtypes.bfloat16))

    nc = _get_nc(causal, use_f32r)

    ident_np = cvt(np.eye(P, dtype=np.float32))
    if causal:
        tri = cvt(np.where(np.arange(P)[None, :] >= np.arange(P)[:, None],
                           np.float32(0.0), np.float32(-8e9)))
    else:
        maskT = cvt(np.clip(8.0 * m.T, -3e38, 3e38))

    in_maps = []
    for b in range(B):
        xqT = cvt(Q[b].T)
        xkT = cvt(K[b].T)
        xvT = cvt(V[b].T)
        for g in range(2):
            sl = slice(g * G, (g + 1) * G)
            im = {
                "xqT": xqT, "xkT": xkT, "xvT": xvT,
                "wqT": cvt(np.asarray(wq_w)[sl, :].T),
                "wkT": cvt(np.asarray(wk_w)[sl, :].T),
                "wvT": cvt(np.asarray(wv_w)[sl, :].T),
                "woT": cvt(np.asarray(wo_w)[:, sl].T),
                "bq": np.ascontiguousarray(np.asarray(wq_b)[sl].reshape(G, 1), np.float32),
                "bk": np.ascontiguousarray(np.asarray(wk_b)[sl].reshape(G, 1), np.float32),
                "bv": np.ascontiguousarray(np.asarray(wv_b)[sl].reshape(G, 1), np.float32),
                "bo": (np.ascontiguousarray(np.asarray(wo_b).reshape(D, 1), np.float32)
                       if g == 0 else np.zeros((D, 1), np.float32)),
            }
            im["identity"] = ident_np
            if causal:
                im["tri"] = tri
            else:
                im["maskT"] = maskT
            in_maps.append(im)

    trace = os.environ.get("BASS_KERNEL_TRACE") == "1"
    if trace:
        _install_trace_hook()
    res = run_bass_kernel_spmd(nc, in_maps, core_ids=list(range(NCORES)),
                               trace=trace)
    LAST_RESULTS = res

    outf = np.empty((B, S, D), np.float32)
    for b in range(B):
        part = res.results[2 * b]["out"] + res.results[2 * b + 1]["out"]
        outf[b] = part.T
    return outf


def _install_trace_hook():
    """Register the axon NTFF profiling hook (missing antenv.axon_hooks shim)."""
    import sys
    import types
    import antenv
    if "antenv.axon_hooks" not in sys.modules:
        mod = types.ModuleType("antenv.axon_hooks")
        holder = [None]
        mod.set_axon_ntff_profile_hook = lambda h: holder.__setitem__(0, h)
        mod.get_axon_ntff_profile_hook = lambda: holder[0]
        sys.modules["antenv.axon_hooks"] = mod
        antenv.axon_hooks = mod
        from trn_agent_boot.trn_boot import _ntff_profile_via_ctypes
        mod.set_axon_ntff_profile_hook(
            _ntff_profile_via_ctypes("/opt/axon/libaxon_pjrt.so"))
    import concourse.bass_utils as bu
    bu.upload_artifacts = lambda d: d  # no artifact bucket in this container
